# revision 1
# baseline (speedup 1.0000x reference)
"""Masked graph-attention aggregator on 8 Trainium2 NeuronCores (Bass/Tile).

Computation:
    q/k/v = x @ W{q,k,v}.T + b                     -> [H=8, N=4096, DH=32]
    att   = softmax(mask(q k^T / sqrt(DH)))        mask from edge_index
    y     = att @ v                                -> [N, 256]
    out   = concat([x, y], 1) @ Wp.T + bp          -> [N, 256]

Sharding: query rows split 512 per core; x^T / W^T / per-key query lists are
prepared host-side (pure input relayout+cast), so the device preamble is DMA
only. The V bias is folded host-side into the output-projection bias
(softmax rows sum to 1, so bv passes through attention as a constant add).
Per core:
  - mask^T [4096 keys, 512 queries] built by GPSIMD local_scatter from
    host-deduped per-key lists (int16 local query indices, -1 padded),
    loaded as a single DMA in final SBUF layout; x^T and the projection
    weights are DMA'd first (the scatters finish well before the mask is
    consumed, the projections gate the PE start).
  - K^T/Q^T head-major bf16 projections straight off x^T; K/V projection
    emission is interleaved with group-0 attention chunks so the PE never
    waits for a full projection pass.
  - V packed (via host-permuted Wv columns) per 128-key chunk into
    per-head-pair 128-col PE weight blocks [V_h0 | V_h1 | ones | pad]
    (even groups) or [ones | pad | V_h0 | V_h1] (odd groups): the two
    512-wide PV accumulations per (chunk, pair) land both heads' y^T at
    their final 32-row bands plus a shared denominator row at a 32-aligned
    spare row (one ones-column serves both heads via column ranges).
  - attention per head-pair, per 128-key chunk: S^T on PE (contract 32,
    tile-positioned bands), exp on ACT (1/sqrt(DH) folded into the ACT
    affine), mask multiply on DVE (bf16 2x), PV accumulation on PE.
  - per-group drains: y^T blocks copied at matching bands (DVE), Z row
    staged via ACT, 1/Z by reciprocal_approx_fast (direct from the staged
    row for odd groups whose Z sits at partition 0; via a small DMA hop for
    even groups), broadcast to 32-row bands by one-hot PE matmuls into a
    shared PSUM tile; only the last group's chain sits on the tail.
  - y^T scaling reads the broadcast PSUM directly; final projection in
    bf16 with a rank-1 bias matmul.
  - dependency-free PE warm-up matmuls fill the preamble DMA-wait gap so
    the tensor engine un-throttles before the projections start (they
    scribble on the pv accumulator, which start=True resets at first
    real use).
"""

import numpy as np
import ml_dtypes

import concourse.bass as bass
import concourse.mybir as mybir
import concourse.tile as tile
from concourse import library_config
from concourse.library_overlay import lower_extended_insts

N = 4096
D = 256
H = 8
DH = 32
NCORES = 8
QR = N // NCORES  # 512
W = 24            # per-key in-shard query-list width
NKC = N // 128    # 32 key chunks
SCALE = 1.0 / float(np.sqrt(np.float32(DH)))

f32 = mybir.dt.float32
bf16 = mybir.dt.bfloat16
i16 = mybir.dt.int16

AF = mybir.ActivationFunctionType
OP = mybir.AluOpType

# head order of V columns as produced by the host-permuted WvT:
# [h0 h1 h4 h5 | h2 h3 h6 h7] -> even groups (g0,g2) first, odd (g1,g3) second
VPERM = [0, 1, 4, 5, 2, 3, 6, 7]
# per-kc Vaug block slot for group g (blocks ordered [g0, g2, g1, g3])
GSLOT = [0, 2, 1, 3]


def _split_multi_waits(nc):
    """Walrus encodes at most one sync-wait per instruction; move extras onto
    single-wait NoOps inserted before the instruction on the same engine."""
    ctr = 0
    for f in nc.m.functions:
        for bb in f.blocks:
            il = bb.instructions
            i = 0
            while i < len(il):
                ins = il[i]
                si = ins.sync_info
                if si is not None and len(si.on_wait) > 1:
                    waits = list(si.on_wait)
                    ins.sync_info = mybir.SyncInfo(
                        on_wait=[waits[-1]], on_update=list(si.on_update)
                    )
                    for w in waits[:-1]:
                        ctr += 1
                        nop = mybir.InstNoOp(
                            name=f"I-waitsplit-{ctr}", ins=[], outs=[]
                        )
                        nop.engine = ins.engine
                        nop.sync_info = mybir.SyncInfo(on_wait=[w], on_update=[])
                        il.insert(i, nop)
                        i += 1
                i += 1


def build_program(split: bool = True) -> bass.Bass:
    nc = bass.Bass()

    xT_in = nc.dram_tensor("xT_in", [D, N], bf16, kind="ExternalInput")
    xrT_in = nc.dram_tensor("xrT_in", [D, QR], bf16, kind="ExternalInput")
    wqT_in = nc.dram_tensor("wqT_in", [D, D], bf16, kind="ExternalInput")
    wkT_in = nc.dram_tensor("wkT_in", [D, D], bf16, kind="ExternalInput")
    wvT_in = nc.dram_tensor("wvT_in", [D, D], bf16, kind="ExternalInput")
    wpT_in = nc.dram_tensor("wpT_in", [2 * D, D], bf16, kind="ExternalInput")
    bq_in = nc.dram_tensor("bq_in", [128, 2], f32, kind="ExternalInput")
    bk_in = nc.dram_tensor("bk_in", [128, 2], f32, kind="ExternalInput")
    bp_in = nc.dram_tensor("bp_in", [1, D], bf16, kind="ExternalInput")
    sel_in = nc.dram_tensor("sel_in", [H, H * DH], bf16, kind="ExternalInput")
    klists = nc.dram_tensor("klists", [128, NKC * W], i16, kind="ExternalInput")
    out = nc.dram_tensor("out", [QR, D], f32, kind="ExternalOutput")

    with tile.TileContext(nc) as tc:
        with (
            tc.tile_pool(name="cons", bufs=1) as cons,
            tc.tile_pool(name="big", bufs=1) as big,
            tc.tile_pool(name="work", bufs=3) as work,
        ):
            nc.gpsimd.load_library(library_config.local_scatter)

            xrT = [cons.tile([128, QR], bf16, tag=f"xrT{i}", name=f"xrT{i}") for i in range(2)]
            wqT = [cons.tile([128, D], bf16, tag=f"wqT{i}", name=f"wqT{i}") for i in range(2)]
            xT = [cons.tile([128, N], bf16, tag=f"xT{i}", name=f"xT{i}") for i in range(2)]
            wkT = [cons.tile([128, D], bf16, tag=f"wkT{i}", name=f"wkT{i}") for i in range(2)]
            wvT = [cons.tile([128, D], bf16, tag=f"wvT{i}", name=f"wvT{i}") for i in range(2)]
            wpT = [cons.tile([128, D], bf16, tag=f"wpT{i}", name=f"wpT{i}") for i in range(4)]
            for i in range(2):
                nc.sync.dma_start(out=xrT[i][:], in_=xrT_in[i * 128 : (i + 1) * 128, :])
                nc.sync.dma_start(out=wqT[i][:], in_=wqT_in[i * 128 : (i + 1) * 128, :])
            for i in range(2):
                nc.sync.dma_start(out=wkT[i][:], in_=wkT_in[i * 128 : (i + 1) * 128, :])
                nc.sync.dma_start(out=wvT[i][:], in_=wvT_in[i * 128 : (i + 1) * 128, :])
                nc.sync.dma_start(out=xT[i][:], in_=xT_in[i * 128 : (i + 1) * 128, :])
            # ---------- mask build (after the projection-critical DMAs) ----------
            kl = big.tile([128, NKC * W], i16)
            nc.sync.dma_start(out=kl[:], in_=klists[:])
            ones_w = cons.tile([128, W], bf16)
            nc.vector.memset(ones_w[:], 1.0)
            mask_sb = big.tile([128, NKC * QR], bf16)
            for kc in range(NKC):
                nc.gpsimd.local_scatter(
                    out_ap=mask_sb[:, kc * QR : (kc + 1) * QR],
                    data_ap=ones_w[:],
                    idxs_ap=kl[:, kc * W : (kc + 1) * W],
                    channels=128,
                    num_elems=QR,
                    num_idxs=W,
                )

            for i in range(4):
                nc.sync.dma_start(out=wpT[i][:], in_=wpT_in[i * 128 : (i + 1) * 128, :])
            bq_sb = cons.tile([128, 2], f32)
            bk_sb = cons.tile([128, 2], f32)
            bp_sb = cons.tile([1, D], bf16)
            sel_sb = cons.tile([H, H * DH], bf16)
            nc.sync.dma_start(out=bq_sb[:], in_=bq_in[:])
            nc.sync.dma_start(out=bk_sb[:], in_=bk_in[:])
            nc.sync.dma_start(out=bp_sb[:], in_=bp_in[:])
            nc.sync.dma_start(out=sel_sb[:], in_=sel_in[:])
            ones_bf = cons.tile([1, 128], bf16)
            nc.vector.memset(ones_bf[:], 1.0)

            QT = [big.tile([128, QR], bf16, tag=f"QT{i}", name=f"QT{i}") for i in range(2)]
            KT = [big.tile([128, N], bf16, tag=f"KT{i}", name=f"KT{i}") for i in range(2)]
            Vaug = big.tile([128, NKC * 512], bf16)
            vv = Vaug[:].rearrange("p (kc s c) -> p kc s c", kc=NKC, s=4)
            nc.vector.memset(vv[:, :, 0:2, 64:65], 1.0)
            nc.vector.memset(vv[:, :, 2:4, 0:1], 1.0)

            ystage = [
                cons.tile([128, QR], f32, tag=f"yst{i}", name=f"yst{i}")
                for i in range(2)
            ]
            yT = [cons.tile([128, QR], bf16, tag=f"yt{i}", name=f"yt{i}") for i in range(2)]
            zalle = cons.tile([2, QR], f32, tag="zalle", name="zalle")
            rze = cons.tile([2, QR], f32, tag="rze", name="rze")
            rzbe = cons.tile([2, QR], bf16, tag="rzbe", name="rzbe")
            rz1024 = cons.tile([1, 1024], f32, tag="rz1024", name="rz1024")
            rzb1024 = cons.tile([1, 1024], bf16, tag="rzb1024", name="rzb1024")

            with tc.tile_pool(name="psa", bufs=1, space="PSUM") as psa:
                pvt = [None, None]  # double-rotate pv tiles manually: bufs=1 tag per parity

                def emit_attn(g, kc):
                    h0, h1 = 2 * g, 2 * g + 1
                    dt_ = h0 // 4
                    pv = pvt[0]
                    sp = psa.tile([128, 1024], f32, tag="sp", bufs=2)
                    for j, h in enumerate((h0, h1)):
                        band = (h % 4) * DH
                        nc.tensor.matmul(
                            sp[:, j * 512 : (j + 1) * 512],
                            lhsT=KT[dt_][band : band + DH, kc * 128 : (kc + 1) * 128],
                            rhs=QT[dt_][band : band + DH, :],
                            start=True,
                            stop=True,
                            tile_position=(band, 0),
                        )
                    praw = work.tile([128, 1024], bf16, tag="praw")
                    nc.scalar.activation(praw[:], sp[:], AF.Exp, scale=SCALE)
                    phat = work.tile([128, 1024], bf16, tag="phat")
                    for j in range(2):
                        nc.vector.tensor_tensor(
                            out=phat[:, j * 512 : (j + 1) * 512],
                            in0=praw[:, j * 512 : (j + 1) * 512],
                            in1=mask_sb[:, kc * QR : (kc + 1) * QR],
                            op=OP.mult,
                        )
                    for j in range(2):
                        nc.tensor.matmul(
                            pv[:, j * 512 : (j + 1) * 512],
                            lhsT=Vaug[:, (kc * 4 + GSLOT[g]) * 128 : (kc * 4 + GSLOT[g] + 1) * 128],
                            rhs=phat[:, j * 512 : (j + 1) * 512],
                            start=(kc == 0),
                            stop=(kc == NKC - 1),
                        )

                def drain_group(g, rp):
                    h0, h1 = 2 * g, 2 * g + 1
                    dt_ = h0 // 4
                    band0, band1 = (h0 % 4) * DH, (h1 % 4) * DH
                    zrow = 64 if g % 2 == 0 else 0
                    pv = pvt[0]
                    nc.vector.tensor_copy(
                        ystage[dt_][band0 : band0 + DH, :], pv[band0 : band0 + DH, 0:512]
                    )
                    nc.vector.tensor_copy(
                        ystage[dt_][band1 : band1 + DH, :], pv[band1 : band1 + DH, 512:1024]
                    )
                    zst = work.tile([128, 1024], f32, tag="zst", bufs=2)
                    nc.scalar.activation(
                        zst[zrow : zrow + 1, :], pv[zrow : zrow + 1, :], AF.Copy
                    )
                    if g % 2 == 1:
                        # Z row already at partition 0: direct 1/Z, no DMA hop
                        nc.vector.reciprocal_approx_fast(
                            rz1024[0:1, :], zst[0:1, :]
                        )
                        nc.vector.tensor_copy(rzb1024[0:1, :], rz1024[0:1, :])
                        for j, band in enumerate((band0, band1)):
                            nc.tensor.matmul(
                                rp[band : band + DH, :],
                                lhsT=sel_sb[0:1, 0:DH],
                                rhs=rzb1024[0:1, j * 512 : (j + 1) * 512],
                                start=True,
                                stop=True,
                                tile_position=(0, band),
                            )
                    else:
                        nc.sync.dma_start(
                            out=zalle[0:1, :], in_=zst[zrow : zrow + 1, 0:512]
                        )
                        nc.sync.dma_start(
                            out=zalle[1:2, :], in_=zst[zrow : zrow + 1, 512:1024]
                        )
                        nc.vector.reciprocal_approx_fast(rze[:], zalle[:])
                        nc.vector.tensor_copy(rzbe[:], rze[:])
                        for j, band in enumerate((band0, band1)):
                            nc.tensor.matmul(
                                rp[band : band + DH, :],
                                lhsT=sel_sb[0:2, j * DH : (j + 1) * DH],
                                rhs=rzbe[:],
                                start=True,
                                stop=True,
                                tile_position=(0, band),
                            )

                def dt_chain(dt_, rp):
                    nc.vector.tensor_tensor(
                        out=yT[dt_][:], in0=ystage[dt_][:], in1=rp[:], op=OP.mult
                    )

                with tc.tile_pool(name="psp", bufs=1, space="PSUM") as psp:
                    # PE warm-up: dependency-free matmuls during the DMA wait
                    # keep HAM from throttling the cold start; they scribble
                    # on pv, which is reset by start=True at first real use.
                    pvt[0] = psa.tile([128, 1024], f32, tag="pv", bufs=1, name="pv0")
                    for _ in range(6):
                        nc.tensor.matmul(
                            pvt[0][:, 0:128],
                            lhsT=ones_bf[:, 0:128],
                            rhs=ones_bf[:, 0:128],
                            start=True,
                            stop=True,
                        )
                    # Q projection
                    for dt_ in range(2):
                        qp = psp.tile([128, QR], f32, tag="kp", bufs=1)
                        for cc in range(2):
                            nc.tensor.matmul(
                                qp[:],
                                lhsT=wqT[cc][:, dt_ * 128 : (dt_ + 1) * 128],
                                rhs=xrT[cc][:],
                                start=(cc == 0),
                                stop=(cc == 1),
                            )
                        nc.vector.tensor_scalar(
                            out=QT[dt_][:], in0=qp[:], scalar1=bq_sb[:, dt_ : dt_ + 1],
                            scalar2=None, op0=OP.add,
                        )
                    for _ in range(25):
                        nc.tensor.matmul(
                            pvt[0][:, 0:128],
                            lhsT=ones_bf[:, 0:128],
                            rhs=ones_bf[:, 0:128],
                            start=True,
                            stop=True,
                        )
                    # K/V projections interleaved with group-0 attention
                    for nch in range(8):
                        for dt_ in range(2):
                            kp = psp.tile([128, 512], f32, tag="kp", bufs=1)
                            for cc in range(2):
                                nc.tensor.matmul(
                                    kp[:],
                                    lhsT=wkT[cc][:, dt_ * 128 : (dt_ + 1) * 128],
                                    rhs=xT[cc][:, nch * 512 : (nch + 1) * 512],
                                    start=(cc == 0),
                                    stop=(cc == 1),
                                )
                            nc.vector.tensor_scalar(
                                out=KT[dt_][:, nch * 512 : (nch + 1) * 512], in0=kp[:],
                                scalar1=bk_sb[:, dt_ : dt_ + 1], scalar2=None, op0=OP.add,
                            )
                        for nb in range(nch * 4, nch * 4 + 4):
                            vp = psp.tile([128, D], f32, tag="vp", bufs=1)
                            for cc in range(2):
                                nc.tensor.matmul(
                                    vp[:],
                                    lhsT=xT[cc][:, nb * 128 : (nb + 1) * 128],
                                    rhs=wvT[cc][:],
                                    start=(cc == 0),
                                    stop=(cc == 1),
                                )
                            ks = Vaug[:, nb * 512 : (nb + 1) * 512]
                            nc.vector.tensor_copy(
                                ks.rearrange("p (s c) -> p s c", s=4)[:, 0:2, 0:64],
                                vp[:, 0:128].rearrange("p (s c) -> p s c", s=2),
                            )
                            nc.vector.tensor_copy(
                                ks.rearrange("p (s c) -> p s c", s=4)[:, 2:4, 64:128],
                                vp[:, 128:256].rearrange("p (s c) -> p s c", s=2),
                            )
                        for kc in range(nch * 4, nch * 4 + 4):
                            emit_attn(0, kc)

                with tc.tile_pool(name="psr", bufs=1, space="PSUM") as psr:
                    rp = psr.tile([128, QR], f32, tag="rp", bufs=1, name="rp")
                    drain_group(0, rp)
                    for g in range(1, 4):
                        for kc in range(NKC):
                            emit_attn(g, kc)
                        drain_group(g, rp)
                        if g == 1:
                            dt_chain(0, rp)
                    dt_chain(1, rp)

            # ---------- final projection ----------
            with tc.tile_pool(name="pso", bufs=1, space="PSUM") as pso:
                    catT = [xrT[0], xrT[1], yT[0], yT[1]]
                    for qb in range(4):
                        op_ = pso.tile([128, D], f32, tag="op", bufs=2)
                        for cc in range(4):
                            nc.tensor.matmul(
                                op_[:],
                                lhsT=catT[cc][:, qb * 128 : (qb + 1) * 128],
                                rhs=wpT[cc][:],
                                start=(cc == 0),
                                stop=False,
                            )
                        nc.tensor.matmul(
                            op_[:],
                            lhsT=ones_bf[:, 0:128],
                            rhs=bp_sb[:],
                            start=False,
                            stop=True,
                        )
                        osb = work.tile([128, D], f32, tag="osb")
                        nc.vector.tensor_copy(osb[:], op_[:])
                        nc.sync.dma_start(
                            out=out[qb * 128 : (qb + 1) * 128, :], in_=osb[:]
                        )

    lower_extended_insts(nc)
    if split:
        _split_multi_waits(nc)
    return nc


_PROGRAM = None


def _get_program():
    global _PROGRAM
    if _PROGRAM is None:
        _PROGRAM = build_program()
    return _PROGRAM


def shard_inputs(inputs):
    bf = ml_dtypes.bfloat16
    x = np.asarray(inputs["x"], np.float32)
    ei = np.asarray(inputs["edge_index"])
    src = ei[0].astype(np.int64)
    dst = ei[1].astype(np.int64)
    Wq = np.asarray(inputs["Wq"], np.float32)
    Wk = np.asarray(inputs["Wk"], np.float32)
    Wv = np.asarray(inputs["Wv"], np.float32)
    Wp = np.asarray(inputs["Wp"], np.float32)
    bq = np.asarray(inputs["bq"], np.float32)
    bk = np.asarray(inputs["bk"], np.float32)
    bv = np.asarray(inputs["bv"], np.float32)
    bp = np.asarray(inputs["bp"], np.float32)

    xT = np.ascontiguousarray(x.T.astype(bf))                      # [256, 4096]
    wqT = np.ascontiguousarray(Wq.T.astype(bf))                    # [256, 256]
    wkT = np.ascontiguousarray(Wk.T.astype(bf))
    # Wv columns permuted to [h0 h1 h4 h5 h2 h3 h6 h7] (32-col groups)
    wvT_cols = np.concatenate(
        [Wv.T[:, h * DH : (h + 1) * DH] for h in VPERM], axis=1
    )
    wvT = np.ascontiguousarray(wvT_cols.astype(bf))
    wpT = np.ascontiguousarray(Wp.T.astype(bf))                    # [512, 256]
    bq2 = np.ascontiguousarray(bq.reshape(2, 128).T)               # [128, 2]
    bk2 = np.ascontiguousarray(bk.reshape(2, 128).T)
    # bv in the same permuted head order as wvT
    # softmax rows sum to 1, so the V bias passes through attention as a
    # constant add on y: fold it into the output-projection bias.
    bp_f = bp + bv @ Wp[:, D : 2 * D].T
    bp1 = np.ascontiguousarray(bp_f.reshape(1, D).astype(bf))
    selmat = np.zeros((H, H * DH), bf)
    for h in range(H):
        selmat[h, h * DH : (h + 1) * DH] = 1.0

    in_maps = []
    for c in range(NCORES):
        q0 = c * QR
        selm = (src >= q0) & (src < q0 + QR)
        es = (src[selm] - q0).astype(np.int64)   # local query index
        ed = dst[selm].astype(np.int64)          # key index
        u = np.unique(ed * QR + es)
        ed = u // QR
        es = u % QR
        counts = np.bincount(ed, minlength=N)
        if counts.max() > W:
            raise ValueError(f"per-key list overflow: {counts.max()} > {W}")
        starts = np.concatenate([[0], np.cumsum(counts)[:-1]])
        ranks = np.arange(len(ed)) - starts[ed]
        lists = np.full((N, W), -1, np.int16)
        lists[ed, ranks] = es.astype(np.int16)
        # device SBUF layout: kl[p, kc*W + w] = lists[kc*128 + p, w]
        klh = np.ascontiguousarray(
            lists.reshape(NKC, 128, W).transpose(1, 0, 2).reshape(128, NKC * W)
        )
        in_maps.append(
            {
                "xT_in": xT,
                "xrT_in": np.ascontiguousarray(xT[:, q0 : q0 + QR]),
                "wqT_in": wqT, "wkT_in": wkT, "wvT_in": wvT, "wpT_in": wpT,
                "bq_in": bq2, "bk_in": bk2, "bp_in": bp1,
                "sel_in": selmat,
                "klists": klh,
            }
        )
    return in_maps


def run(inputs, trace=False):
    from concourse.bass_utils import run_bass_kernel_spmd

    nc = _get_program()
    in_maps = shard_inputs(inputs)
    res = run_bass_kernel_spmd(nc, in_maps, core_ids=list(range(NCORES)), trace=trace)
    full = np.concatenate([res.results[c]["out"] for c in range(NCORES)], axis=0)
    return np.ascontiguousarray(full.astype(np.float32)), res


def kernel(**inputs) -> np.ndarray:
    out, _ = run(inputs, trace=False)
    return out

